# revision 15
# baseline (speedup 1.0000x reference)
"""Trainium2 Bass kernel for the exp-kernel multivariate Hawkes process
log-likelihood (B=8, N=2048, D=10).

Strategy (v10)
--------------
Data-parallel over batch: core b computes batch row b fully on-chip and
returns per-event partials [128,17]; the host reduces them and adds the
-T*sum(mu) constant (unshard step).

Host ships per-event GATHERED tables (index lookups + O(N*D) products,
no transcendental math on N):
  bcol[j,r]    = b[r, e_j]                        (au -> u = exp)
  vabarg[j,m]  = -b[e_j,m]*trel_j + ln(ab[e_j,m]) (vab = exp, one Act op)
  negarg[j,d]  = b[d,e_j]*(t_j-T) + ln(a[d,e_j])  (neg part = exp+accum)
so the DVE keeps ONLY the critical chain au -> W -> t1/Q2 -> t2/lamr/lam
plus dw+scan; the Act engine handles every other exponential.  The
negative log-likelihood part is ONE activation with accum_out (a [P,1]
running sum), subtracted from the shipped asumtot on the DVE in a [P,1]
op.  GPSIMD only builds triu during the DMA wait (it contends with the
DVE for the SBUF port, so it gets no steady-state work).

W is built in QUARTERS (4 chunks each) so each prefix-matmul group
starts right after its quarter; the Act PSUM->SBUF bf16 copies feed the
DVE mask+reduce tail group by group.  The gathered inter-chunk state
Sgall is ACCUMULATED by the PE (start=False) on top of the Q2 reduce
results inside PSUM, removing the separate qsum add.

Inter-chunk state S lives in [10_r, (10_m, 16_k)] layout end-to-end:
chunk sums from 16 tiny u^T@onehot matmuls (strided PSUM writes), the
affine recurrence S_{k+1}=d_k(S_k+w_k) is ONE tensor_tensor_scan with
a k=0 decay-reset column, and S is gathered per event with 15 onehotT
matmuls.  A manually emitted InstLoadActFuncSet(id=6) loads the
combined exp+ln table once.
"""
import numpy as np
from contextlib import ExitStack

import ml_dtypes
import concourse.bass as bass
import concourse.mybir as mybir
import concourse.tile as tile
from concourse import bacc
from concourse.bass_utils import run_bass_kernel_spmd
from concourse.masks import make_upper_triangular

f32 = mybir.dt.float32
bf16 = mybir.dt.bfloat16
AL = mybir.AluOpType
AF = mybir.ActivationFunctionType
AX = mybir.AxisListType

P = 128          # partitions == chunk size
KC = 16          # number of chunks
D = 10           # event types
N = P * KC       # 2048 events per batch row
B = 8            # batch == cores
NG = 4           # chunk groups (4 chunks per PSUM bank)

# packed DRAM inputs: name -> (shape, dtype)
INPUTS = {
    "hot_f32": ((P, 176), f32),    # trel(16) bcol(160)
    "ohx": ((P, KC * D * D), bf16),  # onehot expanded over r (1600)
    "hot_bf": ((P, 160), bf16),    # onehot(160)
    "rest_f32": ((P, 340), f32),   # vabarg(160) negarg(160) musub_ev(16)
                                   # asumtot(1) pad(3)
    "oht": ((D, N + 320), bf16),   # onehotT [D,N] | decay args true |
                                   # decay args k0-killed
}


def _body(ctx: ExitStack, tc, ins, out_ap):
    nc = tc.nc
    cpool = ctx.enter_context(tc.tile_pool(name="cpool", bufs=1))
    wpool = ctx.enter_context(tc.tile_pool(name="wpool", bufs=1))
    pp = ctx.enter_context(tc.tile_pool(name="pp", bufs=1, space="PSUM"))

    # one combined exp+ln activation table load (id 6 =
    # natural_log_exp_and_others) emitted first on the Act queue
    nc.scalar.add_instruction(mybir.InstLoadActFuncSet(
        name=nc.get_next_instruction_name(), act_func_set_id=6,
        ins=[], outs=[]))

    # ---- input DMAs spread over the three DMA-capable queues ----
    hot_f32 = cpool.tile([P, 176], f32, tag="hot_f32")
    nc.sync.dma_start(out=hot_f32[:], in_=ins["hot_f32"])
    ohx = cpool.tile([P, KC, D, D], bf16, tag="ohx")
    nc.scalar.dma_start(
        out=ohx[:].rearrange("p c m r -> p (c m r)"), in_=ins["ohx"])
    hot_bf = cpool.tile([P, 160], bf16, tag="hot_bf")
    nc.scalar.dma_start(out=hot_bf[:], in_=ins["hot_bf"])
    rest_f32 = cpool.tile([P, 340], f32, tag="rest_f32")
    nc.gpsimd.dma_start(out=rest_f32[:], in_=ins["rest_f32"])
    oht = cpool.tile([D, N + 320], bf16, tag="oht")
    nc.scalar.dma_start(out=oht[:], in_=ins["oht"])

    trel = hot_f32[:, 0:16]
    bcol = hot_f32[:, 16:176].rearrange("p (c r) -> p c r", c=KC)
    onehot = hot_bf[:, 0:160].rearrange("p (c m) -> p c m", c=KC)
    vabarg = rest_f32[:, 0:160]
    negarg = rest_f32[:, 160:320]
    musub_ev = rest_f32[:, 320:336]
    asumtot = rest_f32[:, 336:337]

    # triu built on-device while the DMAs are in flight (gpsimd is idle)
    triu = wpool.tile([P, P], bf16, tag="triu")
    make_upper_triangular(nc, triu[:], val=1.0, diag=True)

    # ---- critical chain: au -> eu (halves) -> W (quarters) -> prefix ----
    au = wpool.tile([P, KC, D], f32, tag="au")
    nc.vector.tensor_tensor(
        out=au[:], in0=bcol,
        in1=trel.unsqueeze(2).broadcast_to([P, KC, D]), op=AL.mult)
    u = wpool.tile([P, KC, D], bf16, tag="u")
    nc.scalar.activation(u[:, 0:8], au[:, 0:8], AF.Exp)
    nc.scalar.activation(u[:, 8:16], au[:, 8:16], AF.Exp)
    # W[j,(c,m,r)] = [e_j == m] * u[j,r]; quarters so each prefix group
    # can start as soon as its 4 chunks exist.  in0 is the host-expanded
    # onehot so both operands are inner-step-1 bf16 (2x DVE mode).
    W = wpool.tile([P, KC, D, D], bf16, tag="W")
    for q in range(4):
        qs = slice(4 * q, 4 * (q + 1))
        nc.vector.tensor_tensor(
            out=W[:, qs],
            in0=ohx[:, qs],
            in1=u[:, qs].unsqueeze(2).broadcast_to([P, 4, D, D]),
            op=AL.mult)

    # chunk sums straight into scan layout: wsq[r, m, k]
    wsq = pp.tile([D, D, KC], f32, tag="wsq", name="wsq")
    for k in range(KC):
        nc.tensor.matmul(wsq[:, :, k], u[:, k, :], onehot[:, k, :],
                         start=True, stop=True)

    # ---- in-chunk inclusive prefix (PE), one matmul per quarter ----
    Pg = [pp.tile([P, 4, D, D], f32, tag=f"Pg{g}", name=f"Pg{g}")
          for g in range(NG)]
    for g in range(NG):
        nc.tensor.matmul(Pg[g][:], triu[:],
                         W[:, 4 * g:4 * (g + 1)].rearrange(
                             "p c r m -> p (c r m)"),
                         start=True, stop=True)

    # ---- Act: decays + vab/neg exps + per-group PSUM->SBUF copies ----
    decays = wpool.tile([D, 320], f32, tag="decays")
    nc.scalar.activation(decays[:], oht[:, N:N + 320], AF.Exp, scale=-1.0)
    PgSB = wpool.tile([P, KC, D, D], bf16, tag="PgSB")
    vab = wpool.tile([P, KC, D], bf16, tag="vab")
    negex = wpool.tile([P, KC, D], bf16, tag="negex")
    negsum = wpool.tile([P, 1], f32, tag="negsum")
    for g in range(NG):
        gs = slice(4 * g, 4 * (g + 1))
        nc.scalar.copy(PgSB[:, gs], Pg[g][:])
        if g == 1:
            # after two copies: the exps whose consumers run late
            nc.scalar.activation(vab[:].rearrange("p c m -> p (c m)"),
                                 vabarg, AF.Exp)
            nc.scalar.activation(negex[:].rearrange("p c m -> p (c m)"),
                                 negarg, AF.Exp, accum_out=negsum[:])

    # ---- dw + inter-chunk affine scan (DVE; gpsimd cannot) ----
    dw = wpool.tile([D, D, KC], f32, tag="dw")
    nc.vector.tensor_tensor(
        out=dw[:], in0=decays[:, 0:160].rearrange("p (m k) -> p m k", m=D),
        in1=wsq[:], op=AL.mult)
    # S_{k+1} = d0op_k * S_k + d_k*w_k  (d0op kills state at k=0 per m)
    Sout = wpool.tile([D, D * KC], bf16, tag="Sout")
    nc.vector.tensor_tensor_scan(
        Sout[:], decays[:, 160:320],
        dw[:].rearrange("p m k -> p (m k)"), initial=0.0,
        op0=AL.mult, op1=AL.add)
    Soutv = Sout[:].rearrange("p (m k) -> p m k", m=D)

    # ---- gather inter-chunk state per event: Sg[i,m] = S_k[e_i, m] ----
    Sgall = pp.tile([P, KC, D], f32, tag="Sgall", name="Sgall")
    nc.vector.memset(Sgall[:, 0:1, :], 0.0)
    for k in range(1, KC):
        nc.tensor.matmul(Sgall[:, k, :], oht[:, k * P:(k + 1) * P],
                         Soutv[:, :, k - 1], start=True, stop=True)

    # ---- tail: mask by onehot_r and contract r, group by group ----
    t1 = wpool.tile([P, KC, D, D], bf16, tag="t1")
    Q2 = wpool.tile([P, KC, D], f32, tag="Q2")
    for g in range(NG):
        gs = slice(4 * g, 4 * (g + 1))
        nc.vector.tensor_tensor(
            out=t1[:, gs], in0=PgSB[:, gs],
            in1=onehot[:, gs].unsqueeze(2).broadcast_to([P, 4, D, D]),
            op=AL.mult)
        nc.vector.tensor_reduce(out=Q2[:, gs], in_=t1[:, gs],
                                axis=AX.X, op=AL.add)

    # add the gathered inter-chunk state, multiply by vab, contract over m
    qsum = wpool.tile([P, KC, D], f32, tag="qsum")
    nc.vector.tensor_tensor(out=qsum[:], in0=Q2[:], in1=Sgall[:], op=AL.add)
    t2 = wpool.tile([P, KC, D], f32, tag="t2")
    nc.vector.tensor_tensor(out=t2[:], in0=qsum[:], in1=vab[:], op=AL.mult)
    lamr = wpool.tile([P, KC], f32, tag="lamr")
    nc.vector.tensor_reduce(out=lamr[:], in_=t2[:], axis=AX.X, op=AL.add)
    lam = wpool.tile([P, KC], f32, tag="lam")
    nc.vector.tensor_tensor(out=lam[:], in0=lamr[:], in1=musub_ev,
                            op=AL.add)

    lamns = wpool.tile([P, 17], f32, tag="lamns")
    # negative part: sum_{c,m} exp(negarg) - asumtot   ([P,1])
    nc.vector.tensor_tensor(out=lamns[:, 16:17], in0=negsum[:],
                            in1=asumtot, op=AL.subtract)
    nc.scalar.activation(lamns[:, 0:16], lam[:], AF.Ln)
    nc.scalar.dma_start(out=out_ap, in_=lamns[:])


_CACHE = {}


def _build(Tval: float = 0.0):
    key = 0
    if key in _CACHE:
        return _CACHE[key]
    nc = bacc.Bacc("TRN2", target_bir_lowering=False, debug=False)
    ins = {}
    for name, (shape, dt) in INPUTS.items():
        ins[name] = nc.dram_tensor(name, list(shape), dt,
                                   kind="ExternalInput").ap()
    out_ap = nc.dram_tensor("out", [P, 17], f32,
                            kind="ExternalOutput").ap()
    with tile.TileContext(nc) as tc:
        with ExitStack() as ctx:
            _body(ctx, tc, ins, out_ap)
    nc.compile()
    _CACHE[key] = (nc, ins, out_ap)
    return _CACHE[key]


def make_in_maps(time_points, event_types, mu_raw, log_alpha, log_beta, T):
    Tval = float(np.asarray(T))
    tp = np.asarray(time_points, dtype=np.float32)          # [B, N]
    et = np.asarray(event_types).astype(np.int64)           # [B, N]

    # O(D^2) parameter transforms in float64 -> float32
    mu = np.log1p(np.exp(np.float64(mu_raw))).astype(np.float32)
    al = np.log1p(np.exp(np.float64(log_alpha))).astype(np.float32)
    be = np.log1p(np.exp(np.float64(log_beta))).astype(np.float32)
    ab = (al * be).astype(np.float32)
    musub = mu - np.diag(ab)                                # [D]
    asum = al.sum(axis=0)                                   # [D]
    beT = np.ascontiguousarray(be.T)
    lab = np.log(ab).astype(np.float32)                     # ln(alpha*beta)
    laT = np.ascontiguousarray(np.log(al).T.astype(np.float32))

    in_maps = []
    for b in range(B):
        e = et[b]                                           # [N]
        t = tp[b]
        ts = t[::P]                                         # [KC]
        dtb = np.zeros(KC, dtype=np.float32)
        dtb[:-1] = ts[1:] - ts[:-1]

        # [p, c] views (event j = c*128 + p)
        t2 = t.reshape(KC, P).T                             # [P, KC]
        e2 = e.reshape(KC, P).T                             # [P, KC]
        trel = t2 - ts[None, :]                             # [P, KC]
        tau2 = t2 - np.float32(Tval)                        # [P, KC]

        hot_f32 = np.zeros((P, 176), dtype=np.float32)
        hot_f32[:, 0:16] = trel
        hot_f32[:, 16:176] = beT[e2].reshape(P, KC * D)     # bcol

        oh = (e2[:, :, None] == np.arange(D)[None, None, :])
        hot_bf = np.zeros((P, 160), dtype=ml_dtypes.bfloat16)
        hot_bf[:, 0:160] = oh.reshape(P, KC * D)
        # onehot expanded over the trailing r axis for the 2x W build
        ohx = np.broadcast_to(
            oh[:, :, :, None], (P, KC, D, D)).reshape(P, KC * D * D)
        ohx = np.ascontiguousarray(ohx).astype(ml_dtypes.bfloat16)

        # fused exp args (products of gathered tables: O(N*D) muls/adds)
        vabarg = (-be[e2] * trel[:, :, None] + lab[e2]).reshape(P, KC * D)
        negarg = (beT[e2] * tau2[:, :, None] + laT[e2]).reshape(P, KC * D)
        rest_f32 = np.zeros((P, 340), dtype=np.float32)
        rest_f32[:, 0:160] = vabarg
        rest_f32[:, 160:320] = negarg
        rest_f32[:, 320:336] = musub[e2]
        rest_f32[:, 336] = asum[e2].sum(axis=1)             # asumtot

        oht = np.zeros((D, N + 320), dtype=ml_dtypes.bfloat16)
        oht[:, 0:N] = (e[None, :] == np.arange(D)[:, None])
        bdtb = be[:, :, None] * dtb[None, None, :]          # [D, D, KC]
        oht[:, N:N + 160] = bdtb.reshape(D, D * KC)
        bk0 = bdtb.copy()
        bk0[:, :, 0] = 40.0                                 # exp(-40) ~ 0
        oht[:, N + 160:N + 320] = bk0.reshape(D, D * KC)

        in_maps.append({"hot_f32": hot_f32, "ohx": ohx, "hot_bf": hot_bf,
                        "rest_f32": rest_f32, "oht": oht})
    negconst = np.float32(-Tval * mu.astype(np.float64).sum())
    return in_maps, Tval, negconst


def kernel(time_points, event_types, mu_raw, log_alpha, log_beta, T):
    in_maps, Tval, negconst = make_in_maps(
        time_points, event_types, mu_raw, log_alpha, log_beta, T)
    nc, _, _ = _build(Tval)
    res = run_bass_kernel_spmd(nc, in_maps, list(range(B))).results
    out = np.array([res[b]["out"].sum() + negconst for b in range(B)],
                   dtype=np.float32)  # loglam + neg part both summed
    return out


# revision 22
# speedup vs baseline: 1.0263x; 1.0263x over previous
"""Trainium2 Bass kernel for the exp-kernel multivariate Hawkes process
log-likelihood (B=8, N=2048, D=10).

Strategy (v10)
--------------
Data-parallel over batch: core b computes batch row b fully on-chip and
returns per-event partials [128,17]; the host reduces them and adds the
-T*sum(mu) constant (unshard step).

Host ships per-event GATHERED tables (index lookups + O(N*D) products,
no transcendental math on N):
  bcol[j,r]    = b[r, e_j]                        (au -> u = exp)
  vabarg[j,m]  = -b[e_j,m]*trel_j + ln(ab[e_j,m]) (vab = exp, one Act op)
  negarg[j,d]  = b[d,e_j]*(t_j-T) + ln(a[d,e_j])  (neg part = exp+accum)
so the DVE keeps ONLY the critical chain au -> W -> t1/Q2 -> t2/lamr/lam
plus dw+scan; the Act engine handles every other exponential.  The
negative log-likelihood part is ONE activation with accum_out (a [P,1]
running sum), subtracted from the shipped asumtot on the DVE in a [P,1]
op.  GPSIMD only builds triu during the DMA wait (it contends with the
DVE for the SBUF port, so it gets no steady-state work).

W is built in QUARTERS (4 chunks each) so each prefix-matmul group
starts right after its quarter; the Act PSUM->SBUF bf16 copies feed the
DVE mask+reduce tail group by group.  The gathered inter-chunk state
Sgall is ACCUMULATED by the PE (start=False) on top of the Q2 reduce
results inside PSUM, removing the separate qsum add.

Inter-chunk state S lives in [10_r, (10_m, 16_k)] layout end-to-end:
chunk sums from 16 tiny u^T@onehot matmuls (strided PSUM writes), the
affine recurrence S_{k+1}=d_k(S_k+w_k) is ONE tensor_tensor_scan with
a k=0 decay-reset column, and S is gathered per event with 15 onehotT
matmuls.  A manually emitted InstLoadActFuncSet(id=6) loads the
combined exp+ln table once.
"""
import numpy as np
from contextlib import ExitStack

import ml_dtypes
import concourse.bass as bass
import concourse.mybir as mybir
import concourse.tile as tile
from concourse import bacc
from concourse.bass_utils import run_bass_kernel_spmd
from concourse.masks import make_upper_triangular

f32 = mybir.dt.float32
bf16 = mybir.dt.bfloat16
AL = mybir.AluOpType
AF = mybir.ActivationFunctionType
AX = mybir.AxisListType

P = 128          # partitions == chunk size
KC = 16          # number of chunks
D = 10           # event types
N = P * KC       # 2048 events per batch row
B = 8            # batch == cores
NG = 4           # chunk groups (4 chunks per PSUM bank)

# packed DRAM inputs: name -> (shape, dtype)
INPUTS = {
    "hot_f32": ((P, 176), f32),    # trel(16) bcol(160)
    "hot_bf": ((P, 160), bf16),    # onehot(160)
    "rest_f32": ((P, 340), f32),   # vabarg(160) negarg(160) musub_ev(16)
                                   # asumtot(1) pad(3)
    "oht": ((D, N + 320), bf16),   # onehotT [D,N] | decay args true |
                                   # decay args k0-killed
}


def _body(ctx: ExitStack, tc, ins, out_ap):
    nc = tc.nc
    cpool = ctx.enter_context(tc.tile_pool(name="cpool", bufs=1))
    wpool = ctx.enter_context(tc.tile_pool(name="wpool", bufs=1))
    pp = ctx.enter_context(tc.tile_pool(name="pp", bufs=1, space="PSUM"))

    # one combined exp+ln activation table load (id 6 =
    # natural_log_exp_and_others) emitted first on the Act queue
    nc.scalar.add_instruction(mybir.InstLoadActFuncSet(
        name=nc.get_next_instruction_name(), act_func_set_id=6,
        ins=[], outs=[]))

    # ---- input DMAs spread over the three DMA-capable queues ----
    hot_f32 = cpool.tile([P, 176], f32, tag="hot_f32")
    nc.sync.dma_start(out=hot_f32[:], in_=ins["hot_f32"])
    hot_bf = cpool.tile([P, 160], bf16, tag="hot_bf")
    nc.scalar.dma_start(out=hot_bf[:], in_=ins["hot_bf"])
    rest_f32 = cpool.tile([P, 340], f32, tag="rest_f32")
    nc.gpsimd.dma_start(out=rest_f32[:], in_=ins["rest_f32"])
    oht = cpool.tile([D, N + 320], bf16, tag="oht")
    nc.scalar.dma_start(out=oht[:], in_=ins["oht"])

    trel = hot_f32[:, 0:16]
    bcol = hot_f32[:, 16:176].rearrange("p (c r) -> p c r", c=KC)
    onehot = hot_bf[:, 0:160].rearrange("p (c m) -> p c m", c=KC)
    vabarg = rest_f32[:, 0:160]
    negarg = rest_f32[:, 160:320]
    musub_ev = rest_f32[:, 320:336]
    asumtot = rest_f32[:, 336:337]

    # triu built on-device while the DMAs are in flight (gpsimd is idle)
    triu = wpool.tile([P, P], bf16, tag="triu")
    make_upper_triangular(nc, triu[:], val=1.0, diag=True)

    # ---- critical chain: au -> eu (halves) -> W (quarters) -> prefix ----
    au = wpool.tile([P, KC, D], f32, tag="au")
    nc.vector.tensor_tensor(
        out=au[:], in0=bcol,
        in1=trel.unsqueeze(2).broadcast_to([P, KC, D]), op=AL.mult)
    u = wpool.tile([P, KC, D], bf16, tag="u")
    nc.scalar.activation(u[:, 0:8], au[:, 0:8], AF.Exp)
    nc.scalar.activation(u[:, 8:16], au[:, 8:16], AF.Exp)
    # W[j,(c,m,r)] = [e_j == m] * u[j,r]; quarters so each prefix group
    # can start as soon as its 4 chunks exist
    W = wpool.tile([P, KC, D, D], bf16, tag="W")
    for q in range(4):
        qs = slice(4 * q, 4 * (q + 1))
        nc.vector.tensor_tensor(
            out=W[:, qs],
            in0=onehot[:, qs].unsqueeze(3).broadcast_to([P, 4, D, D]),
            in1=u[:, qs].unsqueeze(2).broadcast_to([P, 4, D, D]),
            op=AL.mult)

    # chunk sums straight into scan layout: wsq[r, m, k]
    wsq = pp.tile([D, D, KC], f32, tag="wsq", name="wsq")
    for k in range(KC):
        nc.tensor.matmul(wsq[:, :, k], u[:, k, :], onehot[:, k, :],
                         start=True, stop=True)

    # ---- in-chunk inclusive prefix (PE), one matmul per quarter ----
    Pg = [pp.tile([P, 4, D, D], f32, tag=f"Pg{g}", name=f"Pg{g}")
          for g in range(NG)]
    for g in range(NG):
        nc.tensor.matmul(Pg[g][:], triu[:],
                         W[:, 4 * g:4 * (g + 1)].rearrange(
                             "p c r m -> p (c r m)"),
                         start=True, stop=True)

    # ---- Act: decays + vab exp + per-group PSUM->SBUF copies ----
    decays = wpool.tile([D, 320], f32, tag="decays")
    nc.scalar.activation(decays[:], oht[:, N:N + 320], AF.Exp, scale=-1.0)
    PgSB = wpool.tile([P, KC, D, D], bf16, tag="PgSB")
    vab = wpool.tile([P, KC, D], bf16, tag="vab")
    negex = wpool.tile([P, KC, D], bf16, tag="negex")
    negsum = wpool.tile([P, 1], f32, tag="negsum")
    # vab fits the Act idle window before Pg0 lands; copies then stream
    nc.scalar.activation(vab[:].rearrange("p c m -> p (c m)"),
                         vabarg, AF.Exp)
    for g in range(NG):
        gs = slice(4 * g, 4 * (g + 1))
        nc.scalar.copy(PgSB[:, gs], Pg[g][:])
    # neg part after the copies (its consumer is the final out DMA)
    nc.scalar.activation(negex[:].rearrange("p c m -> p (c m)"),
                         negarg, AF.Exp, accum_out=negsum[:])

    # ---- dw + inter-chunk affine scan (DVE; gpsimd cannot) ----
    dw = wpool.tile([D, D, KC], f32, tag="dw")
    nc.vector.tensor_tensor(
        out=dw[:], in0=decays[:, 0:160].rearrange("p (m k) -> p m k", m=D),
        in1=wsq[:], op=AL.mult)
    # S_{k+1} = d0op_k * S_k + d_k*w_k  (d0op kills state at k=0 per m)
    Sout = wpool.tile([D, D * KC], bf16, tag="Sout")
    nc.vector.tensor_tensor_scan(
        Sout[:], decays[:, 160:320],
        dw[:].rearrange("p m k -> p (m k)"), initial=0.0,
        op0=AL.mult, op1=AL.add)
    Soutv = Sout[:].rearrange("p (m k) -> p m k", m=D)

    # ---- gather inter-chunk state per event: Sg[i,m] = S_k[e_i, m] ----
    Sgall = pp.tile([P, KC, D], f32, tag="Sgall", name="Sgall")
    nc.vector.memset(Sgall[:, 0:1, :], 0.0)
    for k in range(1, KC):
        nc.tensor.matmul(Sgall[:, k, :], oht[:, k * P:(k + 1) * P],
                         Soutv[:, :, k - 1], start=True, stop=True)

    # ---- tail: mask by onehot_r and contract r, in HALVES (8 chunks
    # per op: fewer DVE op overheads; the first half only needs the
    # first two Act copies) ----
    t1 = wpool.tile([P, KC, D, D], bf16, tag="t1")
    Q2 = wpool.tile([P, KC, D], f32, tag="Q2")
    for h in range(2):
        hs = slice(8 * h, 8 * (h + 1))
        nc.vector.tensor_tensor(
            out=t1[:, hs], in0=PgSB[:, hs],
            in1=onehot[:, hs].unsqueeze(2).broadcast_to([P, 8, D, D]),
            op=AL.mult)
        nc.vector.tensor_reduce(out=Q2[:, hs], in_=t1[:, hs],
                                axis=AX.X, op=AL.add)

    # add the gathered inter-chunk state, multiply by vab, contract over m
    qsum = wpool.tile([P, KC, D], f32, tag="qsum")
    nc.vector.tensor_tensor(out=qsum[:], in0=Q2[:], in1=Sgall[:], op=AL.add)
    t2 = wpool.tile([P, KC, D], f32, tag="t2")
    nc.vector.tensor_tensor(out=t2[:], in0=qsum[:], in1=vab[:], op=AL.mult)
    lamr = wpool.tile([P, KC], f32, tag="lamr")
    nc.vector.tensor_reduce(out=lamr[:], in_=t2[:], axis=AX.X, op=AL.add)
    lam = wpool.tile([P, KC], f32, tag="lam")
    nc.vector.tensor_tensor(out=lam[:], in0=lamr[:], in1=musub_ev,
                            op=AL.add)

    lamns = wpool.tile([P, 17], f32, tag="lamns")
    # negative part: sum_{c,m} exp(negarg) - asumtot   ([P,1])
    nc.vector.tensor_tensor(out=lamns[:, 16:17], in0=negsum[:],
                            in1=asumtot, op=AL.subtract)
    nc.scalar.activation(lamns[:, 0:16], lam[:], AF.Ln)
    nc.scalar.dma_start(out=out_ap, in_=lamns[:])


_CACHE = {}


def _build(Tval: float = 0.0):
    key = 0
    if key in _CACHE:
        return _CACHE[key]
    nc = bacc.Bacc("TRN2", target_bir_lowering=False, debug=False)
    ins = {}
    for name, (shape, dt) in INPUTS.items():
        ins[name] = nc.dram_tensor(name, list(shape), dt,
                                   kind="ExternalInput").ap()
    out_ap = nc.dram_tensor("out", [P, 17], f32,
                            kind="ExternalOutput").ap()
    with tile.TileContext(nc) as tc:
        with ExitStack() as ctx:
            _body(ctx, tc, ins, out_ap)
    nc.compile()
    _CACHE[key] = (nc, ins, out_ap)
    return _CACHE[key]


def make_in_maps(time_points, event_types, mu_raw, log_alpha, log_beta, T):
    Tval = float(np.asarray(T))
    tp = np.asarray(time_points, dtype=np.float32)          # [B, N]
    et = np.asarray(event_types).astype(np.int64)           # [B, N]

    # O(D^2) parameter transforms in float64 -> float32
    mu = np.log1p(np.exp(np.float64(mu_raw))).astype(np.float32)
    al = np.log1p(np.exp(np.float64(log_alpha))).astype(np.float32)
    be = np.log1p(np.exp(np.float64(log_beta))).astype(np.float32)
    ab = (al * be).astype(np.float32)
    musub = mu - np.diag(ab)                                # [D]
    asum = al.sum(axis=0)                                   # [D]
    beT = np.ascontiguousarray(be.T)
    lab = np.log(ab).astype(np.float32)                     # ln(alpha*beta)
    laT = np.ascontiguousarray(np.log(al).T.astype(np.float32))

    in_maps = []
    for b in range(B):
        e = et[b]                                           # [N]
        t = tp[b]
        ts = t[::P]                                         # [KC]
        dtb = np.zeros(KC, dtype=np.float32)
        dtb[:-1] = ts[1:] - ts[:-1]

        # [p, c] views (event j = c*128 + p)
        t2 = t.reshape(KC, P).T                             # [P, KC]
        e2 = e.reshape(KC, P).T                             # [P, KC]
        trel = t2 - ts[None, :]                             # [P, KC]
        tau2 = t2 - np.float32(Tval)                        # [P, KC]

        hot_f32 = np.zeros((P, 176), dtype=np.float32)
        hot_f32[:, 0:16] = trel
        hot_f32[:, 16:176] = beT[e2].reshape(P, KC * D)     # bcol

        oh = (e2[:, :, None] == np.arange(D)[None, None, :])
        hot_bf = np.zeros((P, 160), dtype=ml_dtypes.bfloat16)
        hot_bf[:, 0:160] = oh.reshape(P, KC * D)

        # fused exp args (products of gathered tables: O(N*D) muls/adds)
        vabarg = (-be[e2] * trel[:, :, None] + lab[e2]).reshape(P, KC * D)
        negarg = (beT[e2] * tau2[:, :, None] + laT[e2]).reshape(P, KC * D)
        rest_f32 = np.zeros((P, 340), dtype=np.float32)
        rest_f32[:, 0:160] = vabarg
        rest_f32[:, 160:320] = negarg
        rest_f32[:, 320:336] = musub[e2]
        rest_f32[:, 336] = asum[e2].sum(axis=1)             # asumtot

        oht = np.zeros((D, N + 320), dtype=ml_dtypes.bfloat16)
        oht[:, 0:N] = (e[None, :] == np.arange(D)[:, None])
        bdtb = be[:, :, None] * dtb[None, None, :]          # [D, D, KC]
        oht[:, N:N + 160] = bdtb.reshape(D, D * KC)
        bk0 = bdtb.copy()
        bk0[:, :, 0] = 40.0                                 # exp(-40) ~ 0
        oht[:, N + 160:N + 320] = bk0.reshape(D, D * KC)

        in_maps.append({"hot_f32": hot_f32, "hot_bf": hot_bf,
                        "rest_f32": rest_f32, "oht": oht})
    negconst = np.float32(-Tval * mu.astype(np.float64).sum())
    return in_maps, Tval, negconst


def kernel(time_points, event_types, mu_raw, log_alpha, log_beta, T):
    in_maps, Tval, negconst = make_in_maps(
        time_points, event_types, mu_raw, log_alpha, log_beta, T)
    nc, _, _ = _build(Tval)
    res = run_bass_kernel_spmd(nc, in_maps, list(range(B))).results
    out = np.array([res[b]["out"].sum() + negconst for b in range(B)],
                   dtype=np.float32)  # loglam + neg part both summed
    return out


# revision 25
# speedup vs baseline: 1.1523x; 1.1227x over previous
"""Trainium2 Bass kernel for the exp-kernel multivariate Hawkes process
log-likelihood (B=8, N=2048, D=10).

Strategy (v10)
--------------
Data-parallel over batch: core b computes batch row b fully on-chip and
returns per-event partials [128,17]; the host reduces them and adds the
-T*sum(mu) constant (unshard step).

Host ships per-event GATHERED tables (index lookups + O(N*D) products,
no transcendental math on N):
  bcol[j,r]    = b[r, e_j]                        (au -> u = exp)
  vabarg[j,m]  = -b[e_j,m]*trel_j + ln(ab[e_j,m]) (vab = exp, one Act op)
  negarg[j,d]  = b[d,e_j]*(t_j-T) + ln(a[d,e_j])  (neg part = exp+accum)
so the DVE keeps ONLY the critical chain au -> W -> t1/Q2 -> t2/lamr/lam
plus dw+scan; the Act engine handles every other exponential.  The
negative log-likelihood part is ONE activation with accum_out (a [P,1]
running sum), subtracted from the shipped asumtot on the DVE in a [P,1]
op.  GPSIMD only builds triu during the DMA wait (it contends with the
DVE for the SBUF port, so it gets no steady-state work).

W is built in QUARTERS (4 chunks each) so each prefix-matmul group
starts right after its quarter; the Act PSUM->SBUF bf16 copies feed the
DVE mask+reduce tail group by group.  The gathered inter-chunk state
Sgall is ACCUMULATED by the PE (start=False) on top of the Q2 reduce
results inside PSUM, removing the separate qsum add.

Inter-chunk state S lives in [10_r, (10_m, 16_k)] layout end-to-end:
chunk sums from 16 tiny u^T@onehot matmuls (strided PSUM writes), the
affine recurrence S_{k+1}=d_k(S_k+w_k) is ONE tensor_tensor_scan with
a k=0 decay-reset column, and S is gathered per event with 15 onehotT
matmuls.  A manually emitted InstLoadActFuncSet(id=6) loads the
combined exp+ln table once.
"""
import numpy as np
from contextlib import ExitStack

import ml_dtypes
import concourse.bass as bass
import concourse.mybir as mybir
import concourse.tile as tile
from concourse import bacc
from concourse.bass_utils import run_bass_kernel_spmd
from concourse.masks import make_upper_triangular

f32 = mybir.dt.float32
bf16 = mybir.dt.bfloat16
AL = mybir.AluOpType
AF = mybir.ActivationFunctionType
AX = mybir.AxisListType

P = 128          # partitions == chunk size
KC = 16          # number of chunks
D = 10           # event types
N = P * KC       # 2048 events per batch row
B = 8            # batch == cores
NG = 4           # chunk groups (4 chunks per PSUM bank)

# packed DRAM inputs: name -> (shape, dtype)
INPUTS = {
    "hot_f32": ((P, 176), f32),    # trel(16) bcol(160)
    "hot_bf": ((P, 160), bf16),    # onehot(160)
    "rest_f32": ((P, 340), f32),   # vabarg(160) negarg(160) musub_ev(16)
                                   # asumtot(1) pad(3)
    "oht": ((D, N + 320), bf16),   # onehotT [D,N] | decay args true |
                                   # decay args k0-killed
}


def _body(ctx: ExitStack, tc, ins, out_ap):
    nc = tc.nc
    cpool = ctx.enter_context(tc.tile_pool(name="cpool", bufs=1))
    wpool = ctx.enter_context(tc.tile_pool(name="wpool", bufs=1))
    pp = ctx.enter_context(tc.tile_pool(name="pp", bufs=1, space="PSUM"))

    # one combined exp+ln activation table load (id 6 =
    # natural_log_exp_and_others) emitted first on the Act queue
    nc.scalar.add_instruction(mybir.InstLoadActFuncSet(
        name=nc.get_next_instruction_name(), act_func_set_id=6,
        ins=[], outs=[]))

    # ---- input DMAs spread over the three DMA-capable queues ----
    hot_f32 = cpool.tile([P, 176], f32, tag="hot_f32")
    nc.sync.dma_start(out=hot_f32[:], in_=ins["hot_f32"])
    hot_bf = cpool.tile([P, 160], bf16, tag="hot_bf")
    nc.scalar.dma_start(out=hot_bf[:], in_=ins["hot_bf"])
    rest_f32 = cpool.tile([P, 340], f32, tag="rest_f32")
    nc.gpsimd.dma_start(out=rest_f32[:], in_=ins["rest_f32"])
    oht = cpool.tile([D, N + 320], bf16, tag="oht")
    nc.scalar.dma_start(out=oht[:], in_=ins["oht"])

    trel = hot_f32[:, 0:16]
    bcol = hot_f32[:, 16:176].rearrange("p (c r) -> p c r", c=KC)
    onehot = hot_bf[:, 0:160].rearrange("p (c m) -> p c m", c=KC)
    vabarg = rest_f32[:, 0:160]
    negarg = rest_f32[:, 160:320]
    musub_ev = rest_f32[:, 320:336]
    asumtot = rest_f32[:, 336:337]

    # triu built on-device while the DMAs are in flight (gpsimd is idle)
    triu = wpool.tile([P, P], bf16, tag="triu")
    make_upper_triangular(nc, triu[:], val=1.0, diag=True)

    # ---- critical chain: au -> eu (halves) -> W (quarters) -> prefix ----
    au = wpool.tile([P, KC, D], f32, tag="au")
    nc.vector.tensor_tensor(
        out=au[:], in0=bcol,
        in1=trel.unsqueeze(2).broadcast_to([P, KC, D]), op=AL.mult)
    u = wpool.tile([P, KC, D], bf16, tag="u")
    nc.scalar.activation(u[:, 0:8], au[:, 0:8], AF.Exp)
    nc.scalar.activation(u[:, 8:16], au[:, 8:16], AF.Exp)
    # decays + vab right here: they fill the Act window before Pg0 lands
    decays = wpool.tile([D, 320], f32, tag="decays")
    nc.scalar.activation(decays[:], oht[:, N:N + 320], AF.Exp, scale=-1.0)
    vab = wpool.tile([P, KC, D], bf16, tag="vab")
    nc.scalar.activation(vab[:].rearrange("p c m -> p (c m)"),
                         vabarg, AF.Exp)
    # W[j,(c,m,r)] = [e_j == m] * u[j,r]; quarters so each prefix group
    # can start as soon as its 4 chunks exist
    W = wpool.tile([P, KC, D, D], bf16, tag="W")
    for q in range(4):
        qs = slice(4 * q, 4 * (q + 1))
        nc.vector.tensor_tensor(
            out=W[:, qs],
            in0=onehot[:, qs].unsqueeze(3).broadcast_to([P, 4, D, D]),
            in1=u[:, qs].unsqueeze(2).broadcast_to([P, 4, D, D]),
            op=AL.mult)

    # chunk sums straight into scan layout: wsq[r, m, k]
    wsq = pp.tile([D, D, KC], f32, tag="wsq", name="wsq")
    for k in range(KC):
        nc.tensor.matmul(wsq[:, :, k], u[:, k, :], onehot[:, k, :],
                         start=True, stop=True)

    # ---- in-chunk inclusive prefix (PE), one matmul per quarter ----
    Pg = [pp.tile([P, 4, D, D], f32, tag=f"Pg{g}", name=f"Pg{g}")
          for g in range(NG)]
    for g in range(NG):
        nc.tensor.matmul(Pg[g][:], triu[:],
                         W[:, 4 * g:4 * (g + 1)].rearrange(
                             "p c r m -> p (c r m)"),
                         start=True, stop=True)

    # ---- Act: per-group PSUM->SBUF copies; neg exp pushed late via a
    # scheduler wait hint so it cannot delay the copies ----
    PgSB = wpool.tile([P, KC, D, D], bf16, tag="PgSB")
    negex = wpool.tile([P, KC, D], bf16, tag="negex")
    negsum = wpool.tile([P, 1], f32, tag="negsum")
    for g in range(NG):
        gs = slice(4 * g, 4 * (g + 1))
        nc.scalar.copy(PgSB[:, gs], Pg[g][:])
    # neg part after the copies (its consumer is the final out DMA)
    with tc.tile_wait_until(0.008):
        nc.scalar.activation(negex[:].rearrange("p c m -> p (c m)"),
                             negarg, AF.Exp, accum_out=negsum[:])

    # ---- dw + inter-chunk affine scan (DVE; gpsimd cannot) ----
    dw = wpool.tile([D, D, KC], f32, tag="dw")
    nc.vector.tensor_tensor(
        out=dw[:], in0=decays[:, 0:160].rearrange("p (m k) -> p m k", m=D),
        in1=wsq[:], op=AL.mult)
    # S_{k+1} = d0op_k * S_k + d_k*w_k  (d0op kills state at k=0 per m)
    Sout = wpool.tile([D, D * KC], bf16, tag="Sout")
    nc.vector.tensor_tensor_scan(
        Sout[:], decays[:, 160:320],
        dw[:].rearrange("p m k -> p (m k)"), initial=0.0,
        op0=AL.mult, op1=AL.add)
    Soutv = Sout[:].rearrange("p (m k) -> p m k", m=D)

    # ---- gather inter-chunk state per event: Sg[i,m] = S_k[e_i, m] ----
    Sgall = pp.tile([P, KC, D], f32, tag="Sgall", name="Sgall")
    nc.vector.memset(Sgall[:, 0:1, :], 0.0)
    for k in range(1, KC):
        nc.tensor.matmul(Sgall[:, k, :], oht[:, k * P:(k + 1) * P],
                         Soutv[:, :, k - 1], start=True, stop=True)

    # ---- tail: mask by onehot_r and contract r.  Splits (g0, g1,
    # g2+g3): the first op only waits for the first Act copy, the
    # merged back half amortizes DVE op overheads ----
    t1 = wpool.tile([P, KC, D, D], bf16, tag="t1")
    Q2 = wpool.tile([P, KC, D], f32, tag="Q2")
    for hs in (slice(0, 4), slice(4, 8), slice(8, 16)):
        w = hs.stop - hs.start
        nc.vector.tensor_tensor(
            out=t1[:, hs], in0=PgSB[:, hs],
            in1=onehot[:, hs].unsqueeze(2).broadcast_to([P, w, D, D]),
            op=AL.mult)
        nc.vector.tensor_reduce(out=Q2[:, hs], in_=t1[:, hs],
                                axis=AX.X, op=AL.add)

    # add the gathered inter-chunk state, multiply by vab, contract over m
    qsum = wpool.tile([P, KC, D], f32, tag="qsum")
    nc.vector.tensor_tensor(out=qsum[:], in0=Q2[:], in1=Sgall[:], op=AL.add)
    t2 = wpool.tile([P, KC, D], f32, tag="t2")
    nc.vector.tensor_tensor(out=t2[:], in0=qsum[:], in1=vab[:], op=AL.mult)
    lamr = wpool.tile([P, KC], f32, tag="lamr")
    nc.vector.tensor_reduce(out=lamr[:], in_=t2[:], axis=AX.X, op=AL.add)
    lam = wpool.tile([P, KC], f32, tag="lam")
    nc.vector.tensor_tensor(out=lam[:], in0=lamr[:], in1=musub_ev,
                            op=AL.add)

    lamns = wpool.tile([P, 17], f32, tag="lamns")
    # negative part: sum_{c,m} exp(negarg) - asumtot   ([P,1])
    nc.vector.tensor_tensor(out=lamns[:, 16:17], in0=negsum[:],
                            in1=asumtot, op=AL.subtract)
    nc.scalar.activation(lamns[:, 0:16], lam[:], AF.Ln)
    nc.scalar.dma_start(out=out_ap, in_=lamns[:])


_CACHE = {}


def _build(Tval: float = 0.0):
    key = 0
    if key in _CACHE:
        return _CACHE[key]
    nc = bacc.Bacc("TRN2", target_bir_lowering=False, debug=False)
    ins = {}
    for name, (shape, dt) in INPUTS.items():
        ins[name] = nc.dram_tensor(name, list(shape), dt,
                                   kind="ExternalInput").ap()
    out_ap = nc.dram_tensor("out", [P, 17], f32,
                            kind="ExternalOutput").ap()
    with tile.TileContext(nc) as tc:
        with ExitStack() as ctx:
            _body(ctx, tc, ins, out_ap)
    nc.compile()
    _CACHE[key] = (nc, ins, out_ap)
    return _CACHE[key]


def make_in_maps(time_points, event_types, mu_raw, log_alpha, log_beta, T):
    Tval = float(np.asarray(T))
    tp = np.asarray(time_points, dtype=np.float32)          # [B, N]
    et = np.asarray(event_types).astype(np.int64)           # [B, N]

    # O(D^2) parameter transforms in float64 -> float32
    mu = np.log1p(np.exp(np.float64(mu_raw))).astype(np.float32)
    al = np.log1p(np.exp(np.float64(log_alpha))).astype(np.float32)
    be = np.log1p(np.exp(np.float64(log_beta))).astype(np.float32)
    ab = (al * be).astype(np.float32)
    musub = mu - np.diag(ab)                                # [D]
    asum = al.sum(axis=0)                                   # [D]
    beT = np.ascontiguousarray(be.T)
    lab = np.log(ab).astype(np.float32)                     # ln(alpha*beta)
    laT = np.ascontiguousarray(np.log(al).T.astype(np.float32))

    in_maps = []
    for b in range(B):
        e = et[b]                                           # [N]
        t = tp[b]
        ts = t[::P]                                         # [KC]
        dtb = np.zeros(KC, dtype=np.float32)
        dtb[:-1] = ts[1:] - ts[:-1]

        # [p, c] views (event j = c*128 + p)
        t2 = t.reshape(KC, P).T                             # [P, KC]
        e2 = e.reshape(KC, P).T                             # [P, KC]
        trel = t2 - ts[None, :]                             # [P, KC]
        tau2 = t2 - np.float32(Tval)                        # [P, KC]

        hot_f32 = np.zeros((P, 176), dtype=np.float32)
        hot_f32[:, 0:16] = trel
        hot_f32[:, 16:176] = beT[e2].reshape(P, KC * D)     # bcol

        oh = (e2[:, :, None] == np.arange(D)[None, None, :])
        hot_bf = np.zeros((P, 160), dtype=ml_dtypes.bfloat16)
        hot_bf[:, 0:160] = oh.reshape(P, KC * D)

        # fused exp args (products of gathered tables: O(N*D) muls/adds)
        vabarg = (-be[e2] * trel[:, :, None] + lab[e2]).reshape(P, KC * D)
        negarg = (beT[e2] * tau2[:, :, None] + laT[e2]).reshape(P, KC * D)
        rest_f32 = np.zeros((P, 340), dtype=np.float32)
        rest_f32[:, 0:160] = vabarg
        rest_f32[:, 160:320] = negarg
        rest_f32[:, 320:336] = musub[e2]
        rest_f32[:, 336] = asum[e2].sum(axis=1)             # asumtot

        oht = np.zeros((D, N + 320), dtype=ml_dtypes.bfloat16)
        oht[:, 0:N] = (e[None, :] == np.arange(D)[:, None])
        bdtb = be[:, :, None] * dtb[None, None, :]          # [D, D, KC]
        oht[:, N:N + 160] = bdtb.reshape(D, D * KC)
        bk0 = bdtb.copy()
        bk0[:, :, 0] = 40.0                                 # exp(-40) ~ 0
        oht[:, N + 160:N + 320] = bk0.reshape(D, D * KC)

        in_maps.append({"hot_f32": hot_f32, "hot_bf": hot_bf,
                        "rest_f32": rest_f32, "oht": oht})
    negconst = np.float32(-Tval * mu.astype(np.float64).sum())
    return in_maps, Tval, negconst


def kernel(time_points, event_types, mu_raw, log_alpha, log_beta, T):
    in_maps, Tval, negconst = make_in_maps(
        time_points, event_types, mu_raw, log_alpha, log_beta, T)
    nc, _, _ = _build(Tval)
    res = run_bass_kernel_spmd(nc, in_maps, list(range(B))).results
    out = np.array([res[b]["out"].sum() + negconst for b in range(B)],
                   dtype=np.float32)  # loglam + neg part both summed
    return out
